# revision 13
# baseline (speedup 1.0000x reference)
"""Trainium2 Bass kernel for the e3nn-style 5x5x5 SAME conv (dense_cnn).

Strategy
--------
Data-parallel: 8 shards = 2 batches x 4 x-slabs of 12 output planes each.
Each core gets a zero/halo-padded, channel-first input slab [64, 16, 52, 52]
and produces [64, 12, 48, 48].

On-device the conv is a PSUM-accumulated sum over "units" (ty, zb): a unit
is a [128, 128] weight block applied to a de-interleaved input plane window
(partitions 0-63 even z, 64-127 odd z), covering taps tz = zb + s - p for
input z-parity s and output z-parity p. 75 units per output tile (5 tx x
5 ty x 3 zb).

Mixed precision split (rel-err budget 2e-2, measured ~1.5e-2):
  - 15 high-energy units -- the (tx,ty) columns {center, 4 face neighbors},
    which carry the self-connection and the radius-1 taps -- run as bf16
    K=128 matmuls (N=384), exactly like the all-bf16 baseline.
  - The remaining 60 units run in fp8 e4m3 with perf_mode=DoubleRow:
    K=256 matmuls that pack TWO units per instruction (dim1 of the AP
    selects the unit; its stride is the SBUF offset between the two units'
    plane windows). 60 units -> 31 DoubleRow matmuls at ~1.13 cycles/col.
  Per output tile: 46 matmuls / ~50 N-cycles vs 75 in the bf16 baseline.

fp8 scaling: x*32 and w*1024 are quantized to e4m3 (both < 240 max);
bf16 weights carry the same 2^15 product scale; the PSUM->SBUF copy
multiplies by 2^-15.

The tiny 5x5x5x64x64 kernel build (radial basis x Clebsch-Gordan) is done
on the host in numpy; weights ship as packed fp8/bf16 tensors replicated
to every core.
"""

import math

import numpy as np
import ml_dtypes

import concourse.bass as bass
import concourse.mybir as mybir
from concourse import bacc, bass_utils
from concourse.tile import TileContext

MUL = 16
NB = 4
R = 2.5

N_CORES = 8
PX, PY, PZ = 16, 52, 52          # padded per-core input slab (x, y, z)
OX, OY, OZ = 12, 48, 48          # per-core output region
PLANE = PY * PZ                  # 2704 voxels per x-plane
HPL = PZ // 2                    # 26 z-halves per y row
PLANE_D = PY * HPL               # 1352 cols per de-interleaved plane
OPLANE = OY * OZ                 # 2304 outputs per x-plane
YB = 3                           # y-blocks of 16 rows -> N = 16*24 = 384
YBS = OY // YB
NCOL = YBS * (OZ // 2)           # 384 moving columns per matmul

SCALE_X = 32.0                   # fp8 input scale
SCALE_W = 1024.0                 # fp8 weight scale
INV_SCALE = 1.0 / (SCALE_X * SCALE_W)

# (tx, ty) columns kept in bf16 (self-connection + highest-energy taps).
# 3 columns -> rel err ~1.77e-2 (gate 2e-2); 5 columns gave 1.54e-2.
BF16_COLS = ((2, 2), (1, 2), (3, 2))

ZB_LIST = (0, 2, 4)


def _units_for_tx(tx):
    """fp8 unit pairs and bf16 units for one tx, in device program order.

    An odd fp8 unit count would leave a half-empty DoubleRow matmul; the
    leftover unit is promoted to bf16 instead (cheaper and more accurate).
    """
    fp8 = [(ty, zb) for ty in range(5) for zb in ZB_LIST
           if (tx, ty) not in BF16_COLS]
    fp8.sort(key=lambda u: u[0] * HPL + u[1] // 2)
    bf16 = [(ty, zb) for ty in range(5) for zb in ZB_LIST
            if (tx, ty) in BF16_COLS]
    pairs = [(fp8[i], fp8[i + 1]) for i in range(0, len(fp8) - 1, 2)]
    if len(fp8) % 2:
        pairs.append((fp8[-1], None))
    return pairs, bf16


FP8_PAIRS = [_units_for_tx(tx)[0] for tx in range(5)]   # per-tx pair lists
BF16_UNITS = [_units_for_tx(tx)[1] for tx in range(5)]
N_FP8 = [len(p) for p in FP8_PAIRS]                      # 8,6,3,6,8
N_BF16 = [len(u) for u in BF16_UNITS]                    # 0,3,9,3,0
N_MM = sum(N_FP8) + sum(N_BF16)                          # 46 per group


def _build_k(w000, w011, w101, w110, sc0, sc1):
    """Numpy port of the reference kernel build. Returns [5,5,5,64,64]."""
    s = 2
    c = np.arange(-s, s + 1.0)
    lat = np.stack(np.meshgrid(c, c, c, indexing='ij'), axis=-1)
    norm = np.linalg.norm(lat, axis=-1)
    safe = np.where(norm == 0.0, 1.0, norm)
    nvec = np.where(norm[..., None] > 0.0, lat / safe[..., None], 0.0)
    sh1 = np.sqrt(3.0) * nvec
    values = np.linspace(0.0, R, NB + 2)[1:-1]
    step = R / (NB + 1)
    d = (norm[..., None] - values) / step
    dd = np.clip(d, -1.0 + 1e-9, 1.0 - 1e-9)
    emb = np.where(np.abs(d) < 1.0,
                   1.14136 * np.e ** 2 * np.exp(-1.0 / (1.0 - dd ** 2)), 0.0)
    nlat = 125.0

    r000 = np.einsum('xyzb,buw->xyzuw', emb, w000) / nlat
    r011 = np.einsum('xyzb,buw->xyzuw', emb, w011) / nlat
    r101 = np.einsum('xyzb,buw->xyzuw', emb, w101) / nlat
    r110 = np.einsum('xyzb,buw->xyzuw', emb, w110) / nlat
    eye3 = np.eye(3)
    k00 = r000
    k01 = np.einsum('xyzuw,xyzk->xyzuwk', r011, sh1).reshape(5, 5, 5, MUL, 3 * MUL)
    k11 = np.einsum('xyzuw,ik->xyzuiwk', r101, eye3).reshape(5, 5, 5, 3 * MUL, 3 * MUL)
    k10 = np.einsum('xyzuw,xyzi->xyzuiw', r110, sh1).reshape(5, 5, 5, 3 * MUL, MUL) / np.sqrt(3.0)
    top = np.concatenate([k00, k01], axis=-1)
    bot = np.concatenate([k10, k11], axis=-1)
    k = np.concatenate([top, bot], axis=-2)

    lin00 = sc0 / np.sqrt(float(MUL))
    lin11 = np.einsum('uw,ik->uiwk', sc1 / np.sqrt(float(MUL)), eye3).reshape(3 * MUL, 3 * MUL)
    z16 = np.zeros((MUL, 3 * MUL))
    lin = np.concatenate([
        np.concatenate([lin00, z16], axis=1),
        np.concatenate([z16.T, lin11], axis=1)], axis=0)
    k[2, 2, 2] = lin
    return k


def _pack_weights(k):
    """Returns (wts8 [128, n8*256] e4m3, wtsb [128, nb*128] bf16).

    fp8 slot layout per DoubleRow matmul m, unit slot j:
      wts8[s*64+c, m*256 + j*128 + p*64+o] = q8(k*1024)[tx, ty, zb+s-p][c, o]
    bf16 per unit u: wtsb[s*64+c, u*128 + p*64+o] = bf16(k*2^15)[tx,ty,tz][c,o]
    """
    k8 = np.clip(k * SCALE_W, -240.0, 240.0).astype(
        ml_dtypes.float8_e4m3fn).astype(np.float64)
    kb = (k * 32768.0).astype(ml_dtypes.bfloat16).astype(np.float64)

    w8 = np.zeros((sum(N_FP8), 128, 2, 128))
    wb = np.zeros((sum(N_BF16), 128, 128))
    mi = bi = 0
    for tx in range(5):
        for pair in FP8_PAIRS[tx]:
            for j, u in enumerate(pair):
                if u is None:
                    continue
                ty, zb = u
                for s in range(2):
                    for p in range(2):
                        tz = zb + s - p
                        if 0 <= tz <= 4:
                            w8[mi, s * 64:(s + 1) * 64, j, p * 64:(p + 1) * 64] = \
                                k8[tx, ty, tz]
            mi += 1
        for (ty, zb) in BF16_UNITS[tx]:
            for s in range(2):
                for p in range(2):
                    tz = zb + s - p
                    if 0 <= tz <= 4:
                        wb[bi, s * 64:(s + 1) * 64, p * 64:(p + 1) * 64] = \
                            kb[tx, ty, tz]
            bi += 1
    wts8 = np.ascontiguousarray(
        w8.transpose(1, 0, 2, 3).reshape(128, -1)).astype(ml_dtypes.float8_e4m3fn)
    wtsb = np.ascontiguousarray(
        wb.transpose(1, 0, 2).reshape(128, -1)).astype(ml_dtypes.bfloat16)
    return wts8, wtsb


_NC = None


def _get_nc():
    global _NC
    if _NC is None:
        _NC = _build_nc()
    return _NC


def _build_nc():
    nc = bacc.Bacc("TRN2", target_bir_lowering=False)
    f32 = mybir.dt.float32
    bf16 = mybir.dt.bfloat16
    f8 = mybir.dt.float8e4
    DR = mybir.MatmulPerfMode.DoubleRow

    x8e = nc.dram_tensor("x8e", [64, PX * PLANE_D], f8, kind="ExternalInput")
    x8o = nc.dram_tensor("x8o", [64, PX * PLANE_D], f8, kind="ExternalInput")
    xbe = nc.dram_tensor("xbe", [64, PX * PLANE_D], bf16, kind="ExternalInput")
    xbo = nc.dram_tensor("xbo", [64, PX * PLANE_D], bf16, kind="ExternalInput")
    wts8 = nc.dram_tensor("wts8", [128, sum(N_FP8) * 256], f8, kind="ExternalInput")
    wtsb = nc.dram_tensor("wtsb", [128, sum(N_BF16) * 128], bf16, kind="ExternalInput")
    yout = nc.dram_tensor("yout", [64, OX * OPLANE], f32, kind="ExternalOutput")

    with TileContext(nc) as tc:
        with tc.tile_pool(name="wpool", bufs=1) as wpool, \
             tc.tile_pool(name="x8pool", bufs=7) as x8pool, \
             tc.tile_pool(name="xbpool", bufs=7) as xbpool, \
             tc.tile_pool(name="opool", bufs=3) as opool, \
             tc.tile_pool(name="warm", bufs=1) as warmpool, \
             tc.tile_pool(name="ppool", bufs=4, space="PSUM") as ppool, \
             tc.tile_pool(name="pwarm", bufs=1, space="PSUM") as pwarmpool:

            # PE warm-up: ~4us of dummy matmuls on a zeroed tile so the HAM
            # clock gate reaches 8/8 before the first real matmul (which
            # waits on DMA until ~10.5us; engine init gates the memset to
            # ~7us, so the warmup window is what it is).
            wtile = warmpool.tile([128, 512], bf16, name="wtile")
            nc.vector.memset(wtile[:, :], 0)
            pswarm = pwarmpool.tile([128, 512], f32, name="pswarm")
            for _ in range(14):
                nc.tensor.matmul(pswarm[:, :], wtile[:, 0:128], wtile[:, :],
                                 start=True, stop=True)

            # weight chunks per tx (fp8) + one bf16 chunk, DMA'd in
            # priority order interleaved with the first planes below.
            w8_chunks = []
            w8_off = 0
            for tx in range(5):
                cols = N_FP8[tx] * 256
                wtc = wpool.tile([128, cols], f8, tag="wt8", bufs=5, name="wt8")
                w8_chunks.append((wtc, w8_off))
                w8_off += cols
            wtb = wpool.tile([128, sum(N_BF16) * 128], bf16, name="wtb")

            planes8 = {}
            planesb = {}

            def load_plane8(px):
                pt = x8pool.tile([128, PLANE_D], f8, tag="p8", name="p8")
                base = px * PLANE_D
                nc.sync.dma_start(out=pt[0:64, :], in_=x8e[:, base:base + PLANE_D])
                nc.sync.dma_start(out=pt[64:128, :], in_=x8o[:, base:base + PLANE_D])
                planes8[px] = pt

            def load_planeb(px):
                pt = xbpool.tile([128, PLANE_D], bf16, tag="pb", name="pb")
                base = px * PLANE_D
                nc.sync.dma_start(out=pt[0:64, :], in_=xbe[:, base:base + PLANE_D])
                nc.sync.dma_start(out=pt[64:128, :], in_=xbo[:, base:base + PLANE_D])
                planesb[px] = pt

            def dma_w8(tx):
                wtc, off = w8_chunks[tx]
                nc.sync.dma_start(out=wtc[:, :],
                                  in_=wts8[:, off:off + N_FP8[tx] * 256])

            # startup priority order: everything the first output tile
            # needs, interleaved so matmul 0 can start ~1.5us after the
            # DMA engines come up.
            dma_w8(0)
            load_plane8(0)
            nc.sync.dma_start(out=wtb[:, :], in_=wtsb[:, :])
            load_planeb(0)
            dma_w8(1)
            load_plane8(1)
            load_planeb(1)
            dma_w8(2)
            load_plane8(2)
            load_planeb(2)
            dma_w8(3)
            load_plane8(3)
            load_planeb(3)
            dma_w8(4)
            load_plane8(4)
            load_planeb(4)

            def get_plane8(px):
                if px not in planes8:
                    load_plane8(px)
                return planes8[px]

            def get_planeb(px):
                if px not in planesb:
                    load_planeb(px)
                return planesb[px]

            group_idx = 0
            for xo in range(OX):
                # parity-block layout [c, p, y, h] (z = 2h+p, host
                # de-interleaves) so the two parity copies write disjoint
                # ranges and can run concurrently on vector+scalar
                ostage = opool.tile([64, OPLANE], f32, name="ostage")
                ostv = ostage.rearrange("c (p y h) -> c p y h", p=2, h=OZ // 2)
                for yb in range(YB):
                    y0 = yb * YBS
                    ps_full = ppool.tile([128, 512], f32, name="ps")
                    ps = ps_full[:, :NCOL]

                    def fp8_mms(i):
                        for tx in range(5):
                            pt8 = get_plane8(xo + tx)
                            woff = 0
                            for (ua, ub) in FP8_PAIRS[tx]:
                                ty_a, zb_a = ua
                                base = pt8.offset + (y0 + ty_a) * HPL + zb_a // 2
                                if ub is not None:
                                    ty_b, zb_b = ub
                                    sig = (ty_b - ty_a) * HPL + (zb_b - zb_a) // 2
                                else:
                                    sig = 0  # j=1 slot unused (zero weights)
                                rhs = bass.AP(
                                    tensor=pt8.tensor, offset=base,
                                    ap=[[PLANE_D, 128], [sig, 2],
                                        [HPL, YBS], [1, OZ // 2]])
                                lhsT = w8_chunks[tx][0][:, woff:woff + 256] \
                                    .rearrange("c (j m) -> c j m", j=2)
                                nc.tensor.matmul(
                                    ps[:, :], lhsT, rhs,
                                    start=(i == 0), stop=(i == N_MM - 1),
                                    perf_mode=DR)
                                woff += 256
                                i += 1
                        return i

                    def bf16_mms(i):
                        for tx in range(5):
                            if not N_BF16[tx]:
                                continue
                            ptb = get_planeb(xo + tx)
                            ptv = ptb.rearrange("c (y h) -> c y h", h=HPL)
                            ui = sum(N_BF16[:tx])
                            for (ty, zb) in BF16_UNITS[tx]:
                                rhs = ptv[:, y0 + ty:y0 + ty + YBS,
                                          zb // 2:zb // 2 + OZ // 2]
                                lhsT = wtb[:, ui * 128:(ui + 1) * 128]
                                nc.tensor.matmul(
                                    ps[:, :], lhsT, rhs,
                                    start=(i == 0), stop=(i == N_MM - 1))
                                ui += 1
                                i += 1
                        return i

                    # alternate mode order so consecutive groups join
                    # same-mode (the PE stalls ~300ns on bf16->fp8 switches)
                    if group_idx % 2 == 0:
                        i = bf16_mms(fp8_mms(0))
                    else:
                        i = fp8_mms(bf16_mms(0))
                    assert i == N_MM
                    group_idx += 1

                    psv = ps.rearrange("c (y z) -> c y z", z=OZ // 2)
                    # parity copies on separate engines, disjoint ranges
                    nc.vector.tensor_scalar_mul(
                        ostv[:, 0, y0:y0 + YBS, :], psv[0:64, :, :], INV_SCALE)
                    nc.scalar.mul(
                        ostv[:, 1, y0:y0 + YBS, :], psv[64:128, :, :], INV_SCALE)
                    # stream this y-block out immediately (short tail)
                    HB = OZ // 2
                    for p in range(2):
                        off = p * OY * HB + y0 * HB
                        nc.sync.dma_start(
                            out=yout[:, xo * OPLANE + off:
                                     xo * OPLANE + off + YBS * HB],
                            in_=ostage[:, off:off + YBS * HB])
    nc.finalize()
    return nc


def _prep_inputs(x, wts8, wtsb):
    """Returns per-core in_maps. x: [2,48,48,48,64] float32."""
    x8 = np.clip(x * SCALE_X, -240.0, 240.0).astype(ml_dtypes.float8_e4m3fn)
    xb = x.astype(ml_dtypes.bfloat16)
    in_maps = []
    for core in range(N_CORES):
        n, xs = core // 4, (core % 4) * OX
        slab8 = np.pad(x8[n], ((2, 2), (2, 2), (2, 2), (0, 0)))[xs:xs + PX]
        slabb = np.pad(xb[n], ((2, 2), (2, 2), (2, 2), (0, 0)))[xs:xs + PX]
        c8 = slab8.transpose(3, 0, 1, 2)    # [64,16,52,52]
        cb = slabb.transpose(3, 0, 1, 2)
        in_maps.append({
            "x8e": np.ascontiguousarray(c8[..., 0::2]).reshape(64, PX * PLANE_D),
            "x8o": np.ascontiguousarray(c8[..., 1::2]).reshape(64, PX * PLANE_D),
            "xbe": np.ascontiguousarray(cb[..., 0::2]).reshape(64, PX * PLANE_D),
            "xbo": np.ascontiguousarray(cb[..., 1::2]).reshape(64, PX * PLANE_D),
            "wts8": wts8,
            "wtsb": wtsb,
        })
    return in_maps


def _run(inputs, trace=False):
    x = np.asarray(inputs["x"], np.float32)
    k = _build_k(np.asarray(inputs["w000"], np.float64),
                 np.asarray(inputs["w011"], np.float64),
                 np.asarray(inputs["w101"], np.float64),
                 np.asarray(inputs["w110"], np.float64),
                 np.asarray(inputs["sc0"], np.float64),
                 np.asarray(inputs["sc1"], np.float64))
    wts8, wtsb = _pack_weights(k)
    in_maps = _prep_inputs(x, wts8, wtsb)

    nc = _get_nc()
    res = bass_utils.run_bass_kernel_spmd(
        nc, in_maps, core_ids=list(range(N_CORES)), trace=trace)

    out = np.empty((2, 48, 48, 48, 64), np.float32)
    for core in range(N_CORES):
        n, xs = core // 4, (core % 4) * OX
        oc = res.results[core]["yout"].reshape(64, OX, 2, OY, OZ // 2)
        oz = np.empty((64, OX, OY, OZ), np.float32)
        oz[..., 0::2] = oc[:, :, 0]
        oz[..., 1::2] = oc[:, :, 1]
        out[n, xs:xs + OX] = oz.transpose(1, 2, 3, 0)
    return out, res


def kernel(**inputs):
    out, _ = _run(inputs, trace=False)
    return out
